# revision 42
# baseline (speedup 1.0000x reference)
"""Trainium2 Bass kernel for the brute-force antisymmetrized ResNet.

Math (per walker b):
    feats[i,j] = concat(x1[P1[i]], x2[P2[j]]).reshape(24)    (576 = 24*24 perm pairs)
    y0 = tanh(feats @ W0 + b0)
    y1 = tanh(y0 @ W1 + b1) + y0
    y2 = tanh(y1 @ W2 + b2) + y1
    out[b] = log| sum_{i,j} s1[i] s2[j] (y2 @ Wf + bf) |

Strategy:
  - Data-parallel over the 512 walkers: 64 walkers per NeuronCore x 8 cores.
  - Wire traffic is minimized (the axon transfer path is the dominant cost):
      * x1/x2 ship raw ([12, 128] per core, the only runtime input); the 24
        particle permutations are expanded on device with 0/1 perm matmuls.
      * W0/W1/W2/b* are NEFF inline constants (embedded at compile, DMA'd to
        HBM at model load) - zero per-execution weight traffic.  The compile
        cache is keyed by a hash of the weight bytes.
      * The output is one [128, 128] tile per core: per-(channel, walker)
        sign-weighted sums of y2.
  - First layer is factored: y0pre[b,i,j] = u1[b,i] + u2[b,j]; u1/u2 come from
    two tiny matmuls (24 columns per walker each) and a single broadcast-add
    per 8-walker group builds all 576 rows (b0 rides a ones-row in x1f).
  - Activations live in [channel, row] layout; weights are the stationary
    matmul operand.  Plain fp32 matmuls: fp32r/tf32 rounding is amplified
    catastrophically by the antisymmetrization (|anti| ~ 2.6e-4 vs O(1)
    terms).
  - The sign-weighted per-walker reduction multiplies y2 by a replicated,
    period-doubled +-1 sign row into a single-buffer group accumulator, then
    runs one contiguous 576-row reduce_sum per (walker, half) as soon as a
    walker's two tiles land.  No sign-sorting, no segment pieces.
    (tensor_tensor_reduce would fuse the multiply+reduce, but that opcode
    crashes this runtime with NRT_EXEC_UNIT_UNRECOVERABLE.)
  - The tile loop is software-pipelined one deep (layer-1 matmuls of tile j
    issue before layer-2 matmuls of tile j-1) so the TensorE never waits on
    the tanh/residual chain.
  - Host applies Wf and log|.| in fp64.
"""

import itertools

import numpy as np

N1 = 4
N2 = 4
D = 3
BATCH = 512
NDENSE = 256
NCORES = 8
NPERM = 24                                  # 4!
NPAIR = NPERM * NPERM                       # 576
WALKERS_PER_CORE = BATCH // NCORES          # 64
ROWS_PER_CORE = WALKERS_PER_CORE * NPAIR    # 36864
TILE = 512                                  # matmul moving-dim tile
GROUP_WALKERS = 8                           # walkers per h0-ring group
GROUP_ROWS = GROUP_WALKERS * NPAIR          # 4608 = 9 * TILE
TILES_PER_GROUP = GROUP_ROWS // TILE        # 9
NGROUPS = ROWS_PER_CORE // GROUP_ROWS       # 8
UCOLS = WALKERS_PER_CORE * NPERM            # 1536 u-columns per core
K1 = N1 * D + 1                             # 13: x1 features + ones row (b0)
K2 = N2 * D                                 # 12
FEAT = N1 * D                               # 12
WSHARD = NDENSE // NCORES                   # 32 weight rows per core
CBW = NPERM * FEAT                          # const-blob width: 288
CBROWS = 40                                 # 13 x1w + 12 x2w + 12 pm + 2 sg + pad

# Walkers whose 576 rows are fully written once tile j's slice lands (every
# walker spans exactly two 512-row tiles; its reduce issues after the second).
WDONE = [[] for _ in range(NGROUPS * TILES_PER_GROUP)]
for _w in range(BATCH // NCORES):
    _last = (_w * NPAIR + NPAIR - 1) // TILE
    WDONE[_last].append(_w)


def _perms_and_signs(n):
    P = np.array(list(itertools.permutations(range(n))), dtype=np.int32)
    triu = np.triu(np.ones((n, n), dtype=np.int64), 1)
    inv = np.sum((P[:, :, None] > P[:, None, :]) * triu, axis=(1, 2))
    signs = np.where(inv % 2 == 0, 1.0, -1.0).astype(np.float32)
    return P, signs


_P1, _S1 = _perms_and_signs(N1)
_P2, _S2 = _perms_and_signs(N2)
_SGN = (_S1[:, None] * _S2[None, :]).reshape(NPAIR)  # row sign, (i, j) order


def _build_pmats():
    """0/1 perm matrices, packed: pmats[d, 12*i + m] = 1 iff permuting the
    12-feature vector f by perm i gives f'[m] = f[d] (d = 3*P[i][m//3] + m%3).
    P1 == P2 (same lexicographic S4), so one table serves both x1 and x2."""
    pm = np.zeros((FEAT, NPERM * FEAT), dtype=np.float32)
    for i in range(NPERM):
        for m in range(FEAT):
            pm[3 * _P1[i][m // 3] + m % 3, FEAT * i + m] = 1.0
    return pm


_PMATS = _build_pmats()

_cached = {}
_last_results = None  # BassKernelResults of the most recent run (for profiling)


def _build_nc(with_bias: bool, ngroups: int = NGROUPS, consts=None):
    """Build + compile the 8-core SPMD Tile kernel (cached).

    The weights/derived tables arrive via ``consts`` and are embedded in the
    NEFF as model constants (DMA'd to HBM at load, not per execution).  With
    ``consts=None`` the most recently built module is returned (profiling).
    """
    import hashlib

    if consts is None:
        return _cached[("latest", bool(with_bias), ngroups)]
    h = hashlib.sha1()
    for k in sorted(consts):
        h.update(k.encode())
        h.update(np.ascontiguousarray(consts[k]).tobytes())
    key = (bool(with_bias), ngroups, h.hexdigest())
    if key in _cached:
        return _cached[key]

    import concourse.bacc as bacc
    import concourse.tile as tile
    from concourse import mybir

    FP = mybir.dt.float32
    TANH = mybir.ActivationFunctionType.Tanh
    AXX = mybir.AxisListType.X
    ntiles = ngroups * TILES_PER_GROUP
    nwalk = ngroups * GROUP_WALKERS

    nc = bacc.Bacc(
        "TRN2",
        target_bir_lowering=False,
        debug=False,
        num_devices=NCORES,
    )

    # one runtime input per core: raw walker coords (x1 | x2 side by side so
    # one matmul permutes both).  Weights and derived tables are NEFF
    # constants, DMA'd to HBM at model load.
    xr_d = nc.dram_tensor("xr", [FEAT, 2 * WALKERS_PER_CORE], FP,
                          kind="ExternalInput").ap()
    v_d = nc.dram_tensor("v", [128, 2 * WALKERS_PER_CORE], FP,
                         kind="ExternalOutput").ap()

    w1f_d = nc.inline_tensor(consts["w1"], name="w1c").ap()
    w2f_d = nc.inline_tensor(consts["w2"], name="w2c").ap()
    x1w_d = nc.inline_tensor(consts["x1w"], name="x1wc").ap()
    x2w_d = nc.inline_tensor(consts["x2w"], name="x2wc").ap()
    pm_d = nc.inline_tensor(_PMATS, name="pmc").ap()
    sg_d = nc.inline_tensor(
        np.ascontiguousarray(_SGN.reshape(1, NPAIR)), name="sgc").ap()
    if with_bias:
        b_d = nc.inline_tensor(consts["b12"], name="b12c").ap()

    with tile.TileContext(nc) as tc:
        with (
            tc.tile_pool(name="consts", bufs=1) as cpool,
            tc.tile_pool(name="acts", bufs=3) as apool,
            tc.tile_pool(name="t2s", bufs=2) as tpool,
            tc.tile_pool(name="h0ring", bufs=2) as hpool,
            tc.tile_pool(name="vout", bufs=1) as vpool,
            tc.tile_pool(name="ps", bufs=4, space="PSUM") as pspool,
        ):
            w1a = cpool.tile([128, NDENSE], FP, tag="w1a")
            nc.sync.dma_start(w1a[:], w1f_d[0:128, :])
            w1b = cpool.tile([128, NDENSE], FP, tag="w1b")
            nc.sync.dma_start(w1b[:], w1f_d[128:256, :])
            w2a = cpool.tile([128, NDENSE], FP, tag="w2a")
            nc.sync.dma_start(w2a[:], w2f_d[0:128, :])
            w2b = cpool.tile([128, NDENSE], FP, tag="w2b")
            nc.sync.dma_start(w2b[:], w2f_d[128:256, :])

            # ---- small consts (from the raw input / the gathered blob)
            xr = cpool.tile([FEAT, 2 * WALKERS_PER_CORE], FP, tag="xr")
            nc.sync.dma_start(xr[:], xr_d[:])
            # pm first: it gates the perm matmuls (the DMA queue serializes)
            pm = cpool.tile([FEAT, NPERM * FEAT], FP, tag="pm")
            nc.sync.dma_start(pm[:], pm_d[:])
            x1w = cpool.tile([K1, NDENSE], FP, tag="x1w")
            nc.sync.dma_start(x1w[:], x1w_d[:])
            x2w = cpool.tile([K2, NDENSE], FP, tag="x2w")
            nc.sync.dma_start(x2w[:], x2w_d[:])
            sg = cpool.tile([1, NPAIR], FP, tag="sg")
            nc.sync.dma_start(sg[:], sg_d[:])
            if with_bias:
                bsb = cpool.tile([128, 4], FP, tag="b12")  # b1h0 b1h1 b2h0 b2h1
                nc.sync.dma_start(bsb[:], b_d[:])

            ones = cpool.tile([1, 128], FP, tag="ones")
            nc.gpsimd.memset(ones[:], 1.0)
            sgnt = cpool.tile([128, 2 * NPAIR], FP, tag="sgnt")

            # ---- permutation expansion: x1f/x2f [13/12, (w, i)] on device
            x1f = cpool.tile([K1, UCOLS], FP, tag="x1f")
            x2f = cpool.tile([K2, UCOLS], FP, tag="x2f")
            # ones row (partition 12) carries b0; engines can't address a
            # partition range starting at 12, so set all 13 rows and let the
            # perm-expansion copies overwrite rows 0..11.
            nc.gpsimd.memset(x1f[:], 1.0)
            # one [12x12] matmul per perm permutes BOTH x1 and x2 (their 128
            # walker-columns sit side by side in xr); out lands at partitions
            # 0:12 (engine APs cannot start at partition 12), 8 col-blocks
            # per PSUM tile
            W2C = 2 * WALKERS_PER_CORE  # 128 moving cols per perm matmul
            BLK = 2 * TILE // W2C       # 8 perms per PSUM tile
            psps = []
            for _pi in range((NPERM + BLK - 1) // BLK):
                pspi = pspool.tile([128, 2 * TILE], FP, tag="ps",
                                   name=f"pspi{_pi}")
                psps.append(pspi)
            for i in range(NPERM):
                c, il = divmod(i, BLK)
                nc.tensor.matmul(
                    psps[c][0:FEAT, il * W2C:(il + 1) * W2C],
                    pm[:, i * FEAT:(i + 1) * FEAT],
                    xr[:],
                    start=True, stop=True,
                )
            # two strided copies per PSUM tile (x1 part, x2 part): [12, w, i]
            # views on both sides (psum is (i, w)-major, x1f is (w, i)-major)
            for c in range(len(psps)):
                nblk = min(BLK, NPERM - c * BLK)
                src_all = psps[c][0:FEAT, 0:nblk * W2C].rearrange(
                    "p (i w) -> p w i", i=nblk)
                for k, dst in enumerate((x1f, x2f)):
                    dst_ap = dst[0:FEAT, :].rearrange(
                        "p (w i) -> p w i", i=NPERM
                    )[:, :, c * BLK:c * BLK + nblk]
                    src_ap = src_all[
                        :, k * WALKERS_PER_CORE:(k + 1) * WALKERS_PER_CORE, :]
                    if (c + k) % 2 == 0:
                        nc.vector.tensor_copy(dst_ap, src_ap)
                    else:
                        nc.scalar.copy(dst_ap, src_ap)

            # ---- u1s/u2s: first-layer partials, columns (walker, perm)
            u1s = cpool.tile([128, 2, UCOLS], FP, tag="u1s")
            u2s = cpool.tile([128, 2, UCOLS], FP, tag="u2s")
            vout = vpool.tile([128, 2 * WALKERS_PER_CORE], FP, tag="v")
            # single-buffer group accumulator of sign-weighted y2 rows: each
            # region is re-written a full group after its walker's reduce
            y2g = cpool.tile([128, 2, GROUP_ROWS], FP, tag="y2g")

            h0tiles = {}

            def h0tile(g):
                if g not in h0tiles:
                    h0tiles[g] = hpool.tile(
                        [128, 2, GROUP_ROWS], FP, tag="h0g", name=f"h0g{g}"
                    )
                return h0tiles[g]

            def brd(g, h):
                """One broadcast-add builds all 4608 rows of group g, half h."""
                w0 = g * GROUP_WALKERS
                u1h = u1s[:, h, w0 * NPERM:(w0 + GROUP_WALKERS) * NPERM]
                u2h = u2s[:, h, w0 * NPERM:(w0 + GROUP_WALKERS) * NPERM]
                out_ap = h0tile(g)[:, h, :].rearrange(
                    "p (w i j) -> p w i j", i=NPERM, j=NPERM
                )
                in1 = u1h.rearrange(
                    "p (w i u) -> p w i u", i=NPERM, u=1
                ).broadcast_to([128, GROUP_WALKERS, NPERM, NPERM])
                in2 = u2h.rearrange(
                    "p (w u j) -> p w u j", j=NPERM, u=1
                ).broadcast_to([128, GROUP_WALKERS, NPERM, NPERM])
                nc.vector.tensor_add(out_ap, in1, in2)

            def head(j):
                """tanh0 for tile j, in place in the ring."""
                g, s = divmod(j, TILES_PER_GROUP)
                ap = h0tile(g)[:, :, s * TILE:(s + 1) * TILE]
                nc.scalar.activation(ap, ap, TANH)

            # uneven chunks: the first covers exactly group 0's walkers so
            # the first h0 broadcast-add starts as early as possible
            UCH = [(0, GROUP_WALKERS * NPERM)]
            _c0 = GROUP_WALKERS * NPERM
            while _c0 < UCOLS:
                UCH.append((_c0, min(_c0 + 448, UCOLS)))
                _c0 += 448
            for ci, (clo, chi) in enumerate(UCH):
                cw = chi - clo
                for (usb, xf, xw) in ((u1s, x1f, x1w), (u2s, x2f, x2w)):
                    psu = pspool.tile([128, 2 * TILE], FP, tag="ps")
                    for h in (0, 1):
                        nc.tensor.matmul(
                            psu[:, h * TILE:h * TILE + cw],
                            xw[:, h * 128:(h + 1) * 128],
                            xf[:, clo:chi],
                            start=True, stop=True,
                        )
                    nc.vector.tensor_copy(
                        usb[:, :, clo:chi],
                        psu[:, 0:2 * TILE].rearrange(
                            "p (h r) -> p h r", h=2)[:, :, 0:cw],
                    )
                if ci == 0:
                    brd(0, 0)
                    brd(0, 1)
                    head(0)

            # ---- replicate the sign row to all 128 partitions via matmul,
            # doubled to period 2*576 so any 512-row tile window is one
            # slice.  Issued after the u-phase: sgnt is first read by
            # tail(0), long after these run.
            psg = pspool.tile([128, 2 * TILE], FP, tag="ps")
            nc.tensor.matmul(psg[:, 0:288], ones[:], sg[:, 0:288],
                             start=True, stop=True)
            nc.tensor.matmul(psg[:, 512:800], ones[:], sg[:, 288:576],
                             start=True, stop=True)
            nc.vector.tensor_copy(sgnt[:, 0:288], psg[:, 0:288])
            nc.vector.tensor_copy(sgnt[:, 288:576], psg[:, 512:800])
            nc.vector.tensor_copy(sgnt[:, NPAIR:2 * NPAIR], sgnt[:, 0:NPAIR])

            def tail(j, ps1):
                """tanh1 + residual + layer-2 + tanh2 + signed sums, tile j."""
                g, s = divmod(j, TILES_PER_GROUP)
                h0g = h0tiles[g]
                sl = slice(s * TILE, (s + 1) * TILE)
                t1 = apool.tile([128, 2 * TILE], FP, tag="t1")
                if with_bias:
                    for m in (0, 1):
                        nc.scalar.activation(
                            t1[:, m * TILE:(m + 1) * TILE],
                            ps1[:, m * TILE:(m + 1) * TILE],
                            TANH, bias=bsb[:, m:m + 1],
                        )
                else:
                    nc.scalar.activation(t1[:], ps1[:], TANH)
                # residual 1, in place: t1 <- t1 + tanh0
                nc.vector.tensor_add(
                    t1[:].rearrange("p (h r) -> p h r", h=2),
                    t1[:].rearrange("p (h r) -> p h r", h=2),
                    h0g[:, :, sl],
                )
                ps2 = pspool.tile([128, 2 * TILE], FP, tag="ps")
                for m in (0, 1):
                    nc.tensor.matmul(
                        ps2[:, m * TILE:(m + 1) * TILE],
                        w2a[:, m * 128:(m + 1) * 128],
                        t1[:, 0:TILE],
                        start=True, stop=False,
                    )
                    nc.tensor.matmul(
                        ps2[:, m * TILE:(m + 1) * TILE],
                        w2b[:, m * 128:(m + 1) * 128],
                        t1[:, TILE:2 * TILE],
                        start=False, stop=True,
                    )
                t2 = tpool.tile([128, 2 * TILE], FP, tag="t2")
                if with_bias:
                    for m in (0, 1):
                        nc.scalar.activation(
                            t2[:, m * TILE:(m + 1) * TILE],
                            ps2[:, m * TILE:(m + 1) * TILE],
                            TANH, bias=bsb[:, 2 + m:3 + m],
                        )
                else:
                    nc.scalar.activation(t2[:], ps2[:], TANH)
                # y2 = t2 + t1, in place in t2 (Pool engine: it is otherwise
                # idle, and this keeps the DVE free for the residual/brd ops)
                nc.vector.tensor_add(t2[:], t2[:], t1[:])
                # sign-weight into the group accumulator: y2g = y2 * sgn
                q0 = j * TILE - (j * TILE // NPAIR) * NPAIR
                nc.vector.tensor_mul(
                    y2g[:, :, sl],
                    t2[:].rearrange("p (h r) -> p h r", h=2),
                    sgnt[:, q0:q0 + TILE].rearrange(
                        "p (h r) -> p h r", h=1
                    ).broadcast_to([128, 2, TILE]),
                )
                # per-walker signed sums: one contiguous 576-row reduce per
                # (walker, half) as soon as the walker's rows are complete
                for w in WDONE[j]:
                    wl = w % GROUP_WALKERS
                    for m in (0, 1):
                        nc.vector.reduce_sum(
                            vout[:, m * WALKERS_PER_CORE + w:
                                 m * WALKERS_PER_CORE + w + 1],
                            y2g[:, m, wl * NPAIR:(wl + 1) * NPAIR],
                            axis=AXX,
                        )

            prev = None
            for j in range(ntiles):
                g, s = divmod(j, TILES_PER_GROUP)
                h0g = h0tiles[g]
                sl = slice(s * TILE, (s + 1) * TILE)

                # layer 1: 256 -> 256 (tanh0(j) was issued one iter ago)
                ps1 = pspool.tile([128, 2 * TILE], FP, tag="ps")
                for m in (0, 1):
                    nc.tensor.matmul(
                        ps1[:, m * TILE:(m + 1) * TILE],
                        w1a[:, m * 128:(m + 1) * 128],
                        h0g[:, 0, sl],
                        start=True, stop=False,
                    )
                    nc.tensor.matmul(
                        ps1[:, m * TILE:(m + 1) * TILE],
                        w1b[:, m * 128:(m + 1) * 128],
                        h0g[:, 1, sl],
                        start=False, stop=True,
                    )

                # tanh0 for the NEXT tile, ahead of tail's tanh1/tanh2 in
                # the Act queue so layer-1 of j+1 never waits on it
                if j + 1 < ntiles:
                    head(j + 1)

                if prev is not None:
                    tail(*prev)
                prev = (j, ps1)

                # next group's h0: one broadcast-add per half, after tail so
                # the residual adds aren't queued behind them on the DVE
                if s in (2, 4) and g + 1 < ngroups:
                    brd(g + 1, (s - 2) // 2)

                # drain the first half of the walkers' columns early so the
                # final output DMA is short
                if j == 37 and ngroups == NGROUPS:
                    hw_ = WALKERS_PER_CORE // 2
                    nc.sync.dma_start(v_d[:, 0:hw_], vout[:, 0:hw_])
                    nc.sync.dma_start(
                        v_d[:, WALKERS_PER_CORE:WALKERS_PER_CORE + hw_],
                        vout[:, WALKERS_PER_CORE:WALKERS_PER_CORE + hw_])

            tail(*prev)

            hw_ = WALKERS_PER_CORE // 2
            if ngroups == NGROUPS:
                nc.sync.dma_start(v_d[:, hw_:WALKERS_PER_CORE],
                                  vout[:, hw_:WALKERS_PER_CORE])
                nc.sync.dma_start(v_d[:, WALKERS_PER_CORE + hw_:],
                                  vout[:, WALKERS_PER_CORE + hw_:])
            else:
                nc.sync.dma_start(v_d[:], vout[:])

    nc.compile()
    _cached[key] = nc
    _cached[("latest", bool(with_bias), ngroups)] = nc
    return nc


def _make_in_maps(x1, x2, W0, b0, W1, b1, W2, b2):
    with_bias = bool(np.any(b1) or np.any(b2))
    B = x1.shape[0]
    consts = {
        "w1": np.ascontiguousarray(W1),
        "w2": np.ascontiguousarray(W2),
        "x1w": np.ascontiguousarray(
            np.concatenate([W0[:FEAT], b0[None, :]], axis=0)),
        "x2w": np.ascontiguousarray(W0[FEAT:]),
    }
    if with_bias:
        bm = np.zeros((128, 4), dtype=np.float32)
        bm[:, 0] = b1[0:128]
        bm[:, 1] = b1[128:256]
        bm[:, 2] = b2[0:128]
        bm[:, 3] = b2[128:256]
        consts["b12"] = bm
    x1flat = x1.reshape(B, FEAT)
    x2flat = x2.reshape(B, FEAT)
    in_maps = []
    for c in range(NCORES):
        sl = slice(c * WALKERS_PER_CORE, (c + 1) * WALKERS_PER_CORE)
        in_maps.append({
            "xr": np.ascontiguousarray(
                np.concatenate([x1flat[sl].T, x2flat[sl].T], axis=1)
            ),
        })
    return with_bias, consts, in_maps


def _finish(v_per_core, Wf, bf):
    """per-core v [128, 2*64] -> log|anti| [BATCH] in fp64.

    v[p, m*64 + w] = sum_r sgn[r] * y2[c, r] for channel c = m*128 + p.
    The pair signs sum to exactly 0, so bf drops out of the signed sum.
    """
    out = np.empty((BATCH,), dtype=np.float32)
    wf64 = Wf[:, 0].astype(np.float64)
    for c in range(NCORES):
        v = v_per_core[c].astype(np.float64)
        u = np.concatenate(
            [v[:, 0:WALKERS_PER_CORE], v[:, WALKERS_PER_CORE:]], axis=0
        )  # [256, 64]
        anti = wf64 @ u
        out[c * WALKERS_PER_CORE:(c + 1) * WALKERS_PER_CORE] = np.log(
            np.abs(anti)
        ).astype(np.float32)
    return out


def kernel(x1, x2, W0, b0, W1, b1, W2, b2, Wf, bf):
    from concourse.bass_utils import run_bass_kernel_spmd

    x1 = np.asarray(x1, dtype=np.float32)
    x2 = np.asarray(x2, dtype=np.float32)
    W0 = np.asarray(W0, dtype=np.float32)
    b0 = np.asarray(b0, dtype=np.float32)
    W1 = np.asarray(W1, dtype=np.float32)
    b1 = np.asarray(b1, dtype=np.float32)
    W2 = np.asarray(W2, dtype=np.float32)
    b2 = np.asarray(b2, dtype=np.float32)
    Wf = np.asarray(Wf, dtype=np.float32)
    bf = np.asarray(bf, dtype=np.float32)

    with_bias, consts, in_maps = _make_in_maps(x1, x2, W0, b0, W1, b1, W2, b2)
    nc = _build_nc(with_bias, consts=consts)

    # retry: a first execution right after NEFF load has been seen to fail
    # transiently (NRT_EXEC_UNIT_UNRECOVERABLE); a rerun succeeds
    res = None
    for attempt in range(3):
        try:
            res = run_bass_kernel_spmd(nc, in_maps, list(range(NCORES)))
            break
        except ModuleNotFoundError:
            # BASS_TRACE requested but the NTFF profile hook isn't available
            # in this environment; run untraced instead of failing
            import os

            os.environ["BASS_NEVER_TRACE"] = "1"
        except Exception:
            if attempt == 2:
                raise
    global _last_results
    _last_results = res

    return _finish([res.results[c]["v"] for c in range(NCORES)], Wf, bf)


# revision 43
# speedup vs baseline: 1.0048x; 1.0048x over previous
"""Trainium2 Bass kernel for the brute-force antisymmetrized ResNet.

Math (per walker b):
    feats[i,j] = concat(x1[P1[i]], x2[P2[j]]).reshape(24)    (576 = 24*24 perm pairs)
    y0 = tanh(feats @ W0 + b0)
    y1 = tanh(y0 @ W1 + b1) + y0
    y2 = tanh(y1 @ W2 + b2) + y1
    out[b] = log| sum_{i,j} s1[i] s2[j] (y2 @ Wf + bf) |

Strategy:
  - Data-parallel over the 512 walkers: 64 walkers per NeuronCore x 8 cores.
  - Wire traffic is minimized (the axon transfer path is the dominant cost):
      * x1/x2 ship raw ([12, 128] per core, the only runtime input); the 24
        particle permutations are expanded on device with 0/1 perm matmuls.
      * W0/W1/W2/b* are NEFF inline constants (embedded at compile, DMA'd to
        HBM at model load) - zero per-execution weight traffic.  The compile
        cache is keyed by a hash of the weight bytes.
      * The output is one [128, 128] tile per core: per-(channel, walker)
        sign-weighted sums of y2.
  - First layer is factored: y0pre[b,i,j] = u1[b,i] + u2[b,j]; u1/u2 come from
    two tiny matmuls (24 columns per walker each) and a single broadcast-add
    per 8-walker group builds all 576 rows (b0 rides a ones-row in x1f).
  - Activations live in [channel, row] layout; weights are the stationary
    matmul operand.  Plain fp32 matmuls: fp32r/tf32 rounding is amplified
    catastrophically by the antisymmetrization (|anti| ~ 2.6e-4 vs O(1)
    terms).
  - The sign-weighted per-walker reduction multiplies y2 by a replicated,
    period-doubled +-1 sign row into a single-buffer group accumulator, then
    runs one contiguous 576-row reduce_sum per (walker, half) as soon as a
    walker's two tiles land.  No sign-sorting, no segment pieces.
    (tensor_tensor_reduce would fuse the multiply+reduce, but that opcode
    crashes this runtime with NRT_EXEC_UNIT_UNRECOVERABLE.)
  - The tile loop is software-pipelined one deep (layer-1 matmuls of tile j
    issue before layer-2 matmuls of tile j-1) so the TensorE never waits on
    the tanh/residual chain.
  - Host applies Wf and log|.| in fp64.
"""

import itertools

import numpy as np

N1 = 4
N2 = 4
D = 3
BATCH = 512
NDENSE = 256
NCORES = 8
NPERM = 24                                  # 4!
NPAIR = NPERM * NPERM                       # 576
WALKERS_PER_CORE = BATCH // NCORES          # 64
ROWS_PER_CORE = WALKERS_PER_CORE * NPAIR    # 36864
TILE = 512                                  # matmul moving-dim tile
GROUP_WALKERS = 8                           # walkers per h0-ring group
GROUP_ROWS = GROUP_WALKERS * NPAIR          # 4608 = 9 * TILE
TILES_PER_GROUP = GROUP_ROWS // TILE        # 9
NGROUPS = ROWS_PER_CORE // GROUP_ROWS       # 8
UCOLS = WALKERS_PER_CORE * NPERM            # 1536 u-columns per core
K1 = N1 * D + 1                             # 13: x1 features + ones row (b0)
K2 = N2 * D                                 # 12
FEAT = N1 * D                               # 12
WSHARD = NDENSE // NCORES                   # 32 weight rows per core
CBW = NPERM * FEAT                          # const-blob width: 288
CBROWS = 40                                 # 13 x1w + 12 x2w + 12 pm + 2 sg + pad

# Walkers whose 576 rows are fully written once tile j's slice lands (every
# walker spans exactly two 512-row tiles; its reduce issues after the second).
WDONE = [[] for _ in range(NGROUPS * TILES_PER_GROUP)]
for _w in range(BATCH // NCORES):
    _last = (_w * NPAIR + NPAIR - 1) // TILE
    WDONE[_last].append(_w)


def _perms_and_signs(n):
    P = np.array(list(itertools.permutations(range(n))), dtype=np.int32)
    triu = np.triu(np.ones((n, n), dtype=np.int64), 1)
    inv = np.sum((P[:, :, None] > P[:, None, :]) * triu, axis=(1, 2))
    signs = np.where(inv % 2 == 0, 1.0, -1.0).astype(np.float32)
    return P, signs


_P1, _S1 = _perms_and_signs(N1)
_P2, _S2 = _perms_and_signs(N2)
_SGN = (_S1[:, None] * _S2[None, :]).reshape(NPAIR)  # row sign, (i, j) order


def _build_pmats():
    """0/1 perm matrices, packed: pmats[d, 12*i + m] = 1 iff permuting the
    12-feature vector f by perm i gives f'[m] = f[d] (d = 3*P[i][m//3] + m%3).
    P1 == P2 (same lexicographic S4), so one table serves both x1 and x2."""
    pm = np.zeros((FEAT, NPERM * FEAT), dtype=np.float32)
    for i in range(NPERM):
        for m in range(FEAT):
            pm[3 * _P1[i][m // 3] + m % 3, FEAT * i + m] = 1.0
    return pm


_PMATS = _build_pmats()

_cached = {}
_last_results = None  # BassKernelResults of the most recent run (for profiling)


def _build_nc(with_bias: bool, ngroups: int = NGROUPS, consts=None):
    """Build + compile the 8-core SPMD Tile kernel (cached).

    The weights/derived tables arrive via ``consts`` and are embedded in the
    NEFF as model constants (DMA'd to HBM at load, not per execution).  With
    ``consts=None`` the most recently built module is returned (profiling).
    """
    import hashlib

    if consts is None:
        return _cached[("latest", bool(with_bias), ngroups)]
    h = hashlib.sha1()
    for k in sorted(consts):
        h.update(k.encode())
        h.update(np.ascontiguousarray(consts[k]).tobytes())
    key = (bool(with_bias), ngroups, h.hexdigest())
    if key in _cached:
        return _cached[key]

    import concourse.bacc as bacc
    import concourse.tile as tile
    from concourse import mybir

    FP = mybir.dt.float32
    TANH = mybir.ActivationFunctionType.Tanh
    AXX = mybir.AxisListType.X
    ntiles = ngroups * TILES_PER_GROUP
    nwalk = ngroups * GROUP_WALKERS

    nc = bacc.Bacc(
        "TRN2",
        target_bir_lowering=False,
        debug=False,
        num_devices=NCORES,
    )

    # one runtime input per core: raw walker coords (x1 | x2 side by side so
    # one matmul permutes both).  Weights and derived tables are NEFF
    # constants, DMA'd to HBM at model load.
    xr_d = nc.dram_tensor("xr", [FEAT, 2 * WALKERS_PER_CORE], FP,
                          kind="ExternalInput").ap()
    v_d = nc.dram_tensor("v", [128, 2 * WALKERS_PER_CORE], FP,
                         kind="ExternalOutput").ap()

    w1f_d = nc.inline_tensor(consts["w1"], name="w1c").ap()
    w2f_d = nc.inline_tensor(consts["w2"], name="w2c").ap()
    x1w_d = nc.inline_tensor(consts["x1w"], name="x1wc").ap()
    x2w_d = nc.inline_tensor(consts["x2w"], name="x2wc").ap()
    pm_d = nc.inline_tensor(_PMATS, name="pmc").ap()
    sg_d = nc.inline_tensor(
        np.ascontiguousarray(_SGN.reshape(1, NPAIR)), name="sgc").ap()
    if with_bias:
        b_d = nc.inline_tensor(consts["b12"], name="b12c").ap()

    with tile.TileContext(nc) as tc:
        with (
            tc.tile_pool(name="consts", bufs=1) as cpool,
            tc.tile_pool(name="acts", bufs=3) as apool,
            tc.tile_pool(name="t2s", bufs=2) as tpool,
            tc.tile_pool(name="h0ring", bufs=2) as hpool,
            tc.tile_pool(name="vout", bufs=1) as vpool,
            tc.tile_pool(name="ps", bufs=4, space="PSUM") as pspool,
        ):
            # gating DMAs first (the HWDGE queue serializes at ~625ns per
            # descriptor): xr+pm gate the perm matmuls at ~3us, x1w/x2w gate
            # the u-phase; the four big weight tiles aren't read until the
            # first layer-1 matmul ~30us in.
            xr = cpool.tile([FEAT, 2 * WALKERS_PER_CORE], FP, tag="xr")
            nc.sync.dma_start(xr[:], xr_d[:])
            pm = cpool.tile([FEAT, NPERM * FEAT], FP, tag="pm")
            nc.sync.dma_start(pm[:], pm_d[:])
            x1w = cpool.tile([K1, NDENSE], FP, tag="x1w")
            nc.sync.dma_start(x1w[:], x1w_d[:])
            x2w = cpool.tile([K2, NDENSE], FP, tag="x2w")
            nc.sync.dma_start(x2w[:], x2w_d[:])
            sg = cpool.tile([1, NPAIR], FP, tag="sg")
            nc.sync.dma_start(sg[:], sg_d[:])
            w1a = cpool.tile([128, NDENSE], FP, tag="w1a")
            nc.sync.dma_start(w1a[:], w1f_d[0:128, :])
            w1b = cpool.tile([128, NDENSE], FP, tag="w1b")
            nc.sync.dma_start(w1b[:], w1f_d[128:256, :])
            w2a = cpool.tile([128, NDENSE], FP, tag="w2a")
            nc.sync.dma_start(w2a[:], w2f_d[0:128, :])
            w2b = cpool.tile([128, NDENSE], FP, tag="w2b")
            nc.sync.dma_start(w2b[:], w2f_d[128:256, :])
            if with_bias:
                bsb = cpool.tile([128, 4], FP, tag="b12")  # b1h0 b1h1 b2h0 b2h1
                nc.sync.dma_start(bsb[:], b_d[:])

            ones = cpool.tile([1, 128], FP, tag="ones")
            nc.gpsimd.memset(ones[:], 1.0)
            sgnt = cpool.tile([128, 2 * NPAIR], FP, tag="sgnt")

            # ---- permutation expansion: x1f/x2f [13/12, (w, i)] on device
            x1f = cpool.tile([K1, UCOLS], FP, tag="x1f")
            x2f = cpool.tile([K2, UCOLS], FP, tag="x2f")
            # ones row (partition 12) carries b0; engines can't address a
            # partition range starting at 12, so set all 13 rows and let the
            # perm-expansion copies overwrite rows 0..11.
            nc.gpsimd.memset(x1f[:], 1.0)
            # one [12x12] matmul per perm permutes BOTH x1 and x2 (their 128
            # walker-columns sit side by side in xr); out lands at partitions
            # 0:12 (engine APs cannot start at partition 12), 8 col-blocks
            # per PSUM tile
            W2C = 2 * WALKERS_PER_CORE  # 128 moving cols per perm matmul
            BLK = 2 * TILE // W2C       # 8 perms per PSUM tile
            psps = []
            for _pi in range((NPERM + BLK - 1) // BLK):
                pspi = pspool.tile([128, 2 * TILE], FP, tag="ps",
                                   name=f"pspi{_pi}")
                psps.append(pspi)
            for i in range(NPERM):
                c, il = divmod(i, BLK)
                nc.tensor.matmul(
                    psps[c][0:FEAT, il * W2C:(il + 1) * W2C],
                    pm[:, i * FEAT:(i + 1) * FEAT],
                    xr[:],
                    start=True, stop=True,
                )
            # two strided copies per PSUM tile (x1 part, x2 part): [12, w, i]
            # views on both sides (psum is (i, w)-major, x1f is (w, i)-major)
            for c in range(len(psps)):
                nblk = min(BLK, NPERM - c * BLK)
                src_all = psps[c][0:FEAT, 0:nblk * W2C].rearrange(
                    "p (i w) -> p w i", i=nblk)
                for k, dst in enumerate((x1f, x2f)):
                    dst_ap = dst[0:FEAT, :].rearrange(
                        "p (w i) -> p w i", i=NPERM
                    )[:, :, c * BLK:c * BLK + nblk]
                    src_ap = src_all[
                        :, k * WALKERS_PER_CORE:(k + 1) * WALKERS_PER_CORE, :]
                    if (c + k) % 2 == 0:
                        nc.vector.tensor_copy(dst_ap, src_ap)
                    else:
                        nc.scalar.copy(dst_ap, src_ap)

            # ---- u1s/u2s: first-layer partials, columns (walker, perm)
            u1s = cpool.tile([128, 2, UCOLS], FP, tag="u1s")
            u2s = cpool.tile([128, 2, UCOLS], FP, tag="u2s")
            vout = vpool.tile([128, 2 * WALKERS_PER_CORE], FP, tag="v")
            # single-buffer group accumulator of sign-weighted y2 rows: each
            # region is re-written a full group after its walker's reduce
            y2g = cpool.tile([128, 2, GROUP_ROWS], FP, tag="y2g")

            h0tiles = {}

            def h0tile(g):
                if g not in h0tiles:
                    h0tiles[g] = hpool.tile(
                        [128, 2, GROUP_ROWS], FP, tag="h0g", name=f"h0g{g}"
                    )
                return h0tiles[g]

            def brd(g, h):
                """One broadcast-add builds all 4608 rows of group g, half h."""
                w0 = g * GROUP_WALKERS
                u1h = u1s[:, h, w0 * NPERM:(w0 + GROUP_WALKERS) * NPERM]
                u2h = u2s[:, h, w0 * NPERM:(w0 + GROUP_WALKERS) * NPERM]
                out_ap = h0tile(g)[:, h, :].rearrange(
                    "p (w i j) -> p w i j", i=NPERM, j=NPERM
                )
                in1 = u1h.rearrange(
                    "p (w i u) -> p w i u", i=NPERM, u=1
                ).broadcast_to([128, GROUP_WALKERS, NPERM, NPERM])
                in2 = u2h.rearrange(
                    "p (w u j) -> p w u j", j=NPERM, u=1
                ).broadcast_to([128, GROUP_WALKERS, NPERM, NPERM])
                nc.vector.tensor_add(out_ap, in1, in2)

            def head(j):
                """tanh0 for tile j, in place in the ring."""
                g, s = divmod(j, TILES_PER_GROUP)
                ap = h0tile(g)[:, :, s * TILE:(s + 1) * TILE]
                nc.scalar.activation(ap, ap, TANH)

            # uneven chunks: the first covers exactly group 0's walkers so
            # the first h0 broadcast-add starts as early as possible
            UCH = [(0, GROUP_WALKERS * NPERM)]
            _c0 = GROUP_WALKERS * NPERM
            while _c0 < UCOLS:
                UCH.append((_c0, min(_c0 + 448, UCOLS)))
                _c0 += 448
            for ci, (clo, chi) in enumerate(UCH):
                cw = chi - clo
                for (usb, xf, xw) in ((u1s, x1f, x1w), (u2s, x2f, x2w)):
                    psu = pspool.tile([128, 2 * TILE], FP, tag="ps")
                    for h in (0, 1):
                        nc.tensor.matmul(
                            psu[:, h * TILE:h * TILE + cw],
                            xw[:, h * 128:(h + 1) * 128],
                            xf[:, clo:chi],
                            start=True, stop=True,
                        )
                    nc.vector.tensor_copy(
                        usb[:, :, clo:chi],
                        psu[:, 0:2 * TILE].rearrange(
                            "p (h r) -> p h r", h=2)[:, :, 0:cw],
                    )
                if ci == 0:
                    brd(0, 0)
                    brd(0, 1)
                    head(0)

            # ---- replicate the sign row to all 128 partitions via matmul,
            # doubled to period 2*576 so any 512-row tile window is one
            # slice.  Issued after the u-phase: sgnt is first read by
            # tail(0), long after these run.
            psg = pspool.tile([128, 2 * TILE], FP, tag="ps")
            nc.tensor.matmul(psg[:, 0:288], ones[:], sg[:, 0:288],
                             start=True, stop=True)
            nc.tensor.matmul(psg[:, 512:800], ones[:], sg[:, 288:576],
                             start=True, stop=True)
            nc.vector.tensor_copy(sgnt[:, 0:288], psg[:, 0:288])
            nc.vector.tensor_copy(sgnt[:, 288:576], psg[:, 512:800])
            nc.vector.tensor_copy(sgnt[:, NPAIR:2 * NPAIR], sgnt[:, 0:NPAIR])

            def tail(j, ps1):
                """tanh1 + residual + layer-2 + tanh2 + signed sums, tile j."""
                g, s = divmod(j, TILES_PER_GROUP)
                h0g = h0tiles[g]
                sl = slice(s * TILE, (s + 1) * TILE)
                t1 = apool.tile([128, 2 * TILE], FP, tag="t1")
                if with_bias:
                    for m in (0, 1):
                        nc.scalar.activation(
                            t1[:, m * TILE:(m + 1) * TILE],
                            ps1[:, m * TILE:(m + 1) * TILE],
                            TANH, bias=bsb[:, m:m + 1],
                        )
                else:
                    nc.scalar.activation(t1[:], ps1[:], TANH)
                # residual 1, in place: t1 <- t1 + tanh0
                nc.vector.tensor_add(
                    t1[:].rearrange("p (h r) -> p h r", h=2),
                    t1[:].rearrange("p (h r) -> p h r", h=2),
                    h0g[:, :, sl],
                )
                ps2 = pspool.tile([128, 2 * TILE], FP, tag="ps")
                for m in (0, 1):
                    nc.tensor.matmul(
                        ps2[:, m * TILE:(m + 1) * TILE],
                        w2a[:, m * 128:(m + 1) * 128],
                        t1[:, 0:TILE],
                        start=True, stop=False,
                    )
                    nc.tensor.matmul(
                        ps2[:, m * TILE:(m + 1) * TILE],
                        w2b[:, m * 128:(m + 1) * 128],
                        t1[:, TILE:2 * TILE],
                        start=False, stop=True,
                    )
                t2 = tpool.tile([128, 2 * TILE], FP, tag="t2")
                if with_bias:
                    for m in (0, 1):
                        nc.scalar.activation(
                            t2[:, m * TILE:(m + 1) * TILE],
                            ps2[:, m * TILE:(m + 1) * TILE],
                            TANH, bias=bsb[:, 2 + m:3 + m],
                        )
                else:
                    nc.scalar.activation(t2[:], ps2[:], TANH)
                # y2 = t2 + t1, in place in t2 (Pool engine: it is otherwise
                # idle, and this keeps the DVE free for the residual/brd ops)
                nc.vector.tensor_add(t2[:], t2[:], t1[:])
                # sign-weight into the group accumulator: y2g = y2 * sgn
                q0 = j * TILE - (j * TILE // NPAIR) * NPAIR
                nc.vector.tensor_mul(
                    y2g[:, :, sl],
                    t2[:].rearrange("p (h r) -> p h r", h=2),
                    sgnt[:, q0:q0 + TILE].rearrange(
                        "p (h r) -> p h r", h=1
                    ).broadcast_to([128, 2, TILE]),
                )
                # per-walker signed sums: one contiguous 576-row reduce per
                # (walker, half) as soon as the walker's rows are complete
                for w in WDONE[j]:
                    wl = w % GROUP_WALKERS
                    for m in (0, 1):
                        nc.vector.reduce_sum(
                            vout[:, m * WALKERS_PER_CORE + w:
                                 m * WALKERS_PER_CORE + w + 1],
                            y2g[:, m, wl * NPAIR:(wl + 1) * NPAIR],
                            axis=AXX,
                        )

            prev = None
            for j in range(ntiles):
                g, s = divmod(j, TILES_PER_GROUP)
                h0g = h0tiles[g]
                sl = slice(s * TILE, (s + 1) * TILE)

                # layer 1: 256 -> 256 (tanh0(j) was issued one iter ago)
                ps1 = pspool.tile([128, 2 * TILE], FP, tag="ps")
                for m in (0, 1):
                    nc.tensor.matmul(
                        ps1[:, m * TILE:(m + 1) * TILE],
                        w1a[:, m * 128:(m + 1) * 128],
                        h0g[:, 0, sl],
                        start=True, stop=False,
                    )
                    nc.tensor.matmul(
                        ps1[:, m * TILE:(m + 1) * TILE],
                        w1b[:, m * 128:(m + 1) * 128],
                        h0g[:, 1, sl],
                        start=False, stop=True,
                    )

                # tanh0 for the NEXT tile, ahead of tail's tanh1/tanh2 in
                # the Act queue so layer-1 of j+1 never waits on it
                if j + 1 < ntiles:
                    head(j + 1)

                if prev is not None:
                    tail(*prev)
                prev = (j, ps1)

                # next group's h0: one broadcast-add per half, after tail so
                # the residual adds aren't queued behind them on the DVE
                if s in (2, 4) and g + 1 < ngroups:
                    brd(g + 1, (s - 2) // 2)

                # drain the first half of the walkers' columns early so the
                # final output DMA is short
                if j == 37 and ngroups == NGROUPS:
                    hw_ = WALKERS_PER_CORE // 2
                    nc.sync.dma_start(v_d[:, 0:hw_], vout[:, 0:hw_])
                    nc.sync.dma_start(
                        v_d[:, WALKERS_PER_CORE:WALKERS_PER_CORE + hw_],
                        vout[:, WALKERS_PER_CORE:WALKERS_PER_CORE + hw_])

            tail(*prev)

            hw_ = WALKERS_PER_CORE // 2
            if ngroups == NGROUPS:
                nc.sync.dma_start(v_d[:, hw_:WALKERS_PER_CORE],
                                  vout[:, hw_:WALKERS_PER_CORE])
                nc.sync.dma_start(v_d[:, WALKERS_PER_CORE + hw_:],
                                  vout[:, WALKERS_PER_CORE + hw_:])
            else:
                nc.sync.dma_start(v_d[:], vout[:])

    nc.compile()
    _cached[key] = nc
    _cached[("latest", bool(with_bias), ngroups)] = nc
    return nc


def _make_in_maps(x1, x2, W0, b0, W1, b1, W2, b2):
    with_bias = bool(np.any(b1) or np.any(b2))
    B = x1.shape[0]
    consts = {
        "w1": np.ascontiguousarray(W1),
        "w2": np.ascontiguousarray(W2),
        "x1w": np.ascontiguousarray(
            np.concatenate([W0[:FEAT], b0[None, :]], axis=0)),
        "x2w": np.ascontiguousarray(W0[FEAT:]),
    }
    if with_bias:
        bm = np.zeros((128, 4), dtype=np.float32)
        bm[:, 0] = b1[0:128]
        bm[:, 1] = b1[128:256]
        bm[:, 2] = b2[0:128]
        bm[:, 3] = b2[128:256]
        consts["b12"] = bm
    x1flat = x1.reshape(B, FEAT)
    x2flat = x2.reshape(B, FEAT)
    in_maps = []
    for c in range(NCORES):
        sl = slice(c * WALKERS_PER_CORE, (c + 1) * WALKERS_PER_CORE)
        in_maps.append({
            "xr": np.ascontiguousarray(
                np.concatenate([x1flat[sl].T, x2flat[sl].T], axis=1)
            ),
        })
    return with_bias, consts, in_maps


def _finish(v_per_core, Wf, bf):
    """per-core v [128, 2*64] -> log|anti| [BATCH] in fp64.

    v[p, m*64 + w] = sum_r sgn[r] * y2[c, r] for channel c = m*128 + p.
    The pair signs sum to exactly 0, so bf drops out of the signed sum.
    """
    out = np.empty((BATCH,), dtype=np.float32)
    wf64 = Wf[:, 0].astype(np.float64)
    for c in range(NCORES):
        v = v_per_core[c].astype(np.float64)
        u = np.concatenate(
            [v[:, 0:WALKERS_PER_CORE], v[:, WALKERS_PER_CORE:]], axis=0
        )  # [256, 64]
        anti = wf64 @ u
        out[c * WALKERS_PER_CORE:(c + 1) * WALKERS_PER_CORE] = np.log(
            np.abs(anti)
        ).astype(np.float32)
    return out


def kernel(x1, x2, W0, b0, W1, b1, W2, b2, Wf, bf):
    from concourse.bass_utils import run_bass_kernel_spmd

    x1 = np.asarray(x1, dtype=np.float32)
    x2 = np.asarray(x2, dtype=np.float32)
    W0 = np.asarray(W0, dtype=np.float32)
    b0 = np.asarray(b0, dtype=np.float32)
    W1 = np.asarray(W1, dtype=np.float32)
    b1 = np.asarray(b1, dtype=np.float32)
    W2 = np.asarray(W2, dtype=np.float32)
    b2 = np.asarray(b2, dtype=np.float32)
    Wf = np.asarray(Wf, dtype=np.float32)
    bf = np.asarray(bf, dtype=np.float32)

    with_bias, consts, in_maps = _make_in_maps(x1, x2, W0, b0, W1, b1, W2, b2)
    nc = _build_nc(with_bias, consts=consts)

    # retry: a first execution right after NEFF load has been seen to fail
    # transiently (NRT_EXEC_UNIT_UNRECOVERABLE); a rerun succeeds
    res = None
    for attempt in range(3):
        try:
            res = run_bass_kernel_spmd(nc, in_maps, list(range(NCORES)))
            break
        except ModuleNotFoundError:
            # BASS_TRACE requested but the NTFF profile hook isn't available
            # in this environment; run untraced instead of failing
            import os

            os.environ["BASS_NEVER_TRACE"] = "1"
        except Exception:
            if attempt == 2:
                raise
    global _last_results
    _last_results = res

    return _finish([res.results[c]["v"] for c in range(NCORES)], Wf, bf)
